# revision 7
# baseline (speedup 1.0000x reference)
"""Trainium2 Bass kernel for the DiffsolClassifier model.

Network (per image, NCHW fp32):
    z1 = relu(conv2d(x, W1, b1, k=3, s=2, p=1))   # [8,14,14]
    z2 = relu(conv2d(z1, W2, b2, k=3, s=2, p=1))  # [16,7,7]
    t  = flatten(z2) @ Wfc.T + bfc                # [1]
    p  = clip(1 - exp(-(softplus(t) + 1e-3)), 1e-6, 1-1e-6)
       = (1-k) + k*sigmoid(t),  k = exp(-1e-3)    (clip is a no-op)

Sharding: pure data parallel, batch 65536 split 8192/core across 8 cores.

Per-core mapping (16 outer tiles x 512 images):
  - DMA x tile [512, 784] -> SBUF [128, 3136] (partition p holds images 4p+s).
  - PE-transpose 112-pixel chunks to pixel-major [112, 512] (7 chunks).
  - conv1: each output row oi contracts an 84-pixel window; windows are
    expressed against the aligned 112-pixel chunks with zero-padded
    [112,112] weight matrices (Wa/Wb/Wc), N=512 fp32r matmuls accumulating
    in PSUM. Bias+relu fused into the PSUM->SBUF eviction (ACT/DVE).
  - conv2: z1 row-chunks [112=(ch,col), 512] contract with banded
    [112,112] tap matrices (3 taps -> 3 accumulated matmuls per out row).
  - FC: per out-row [112,1] matmul accumulated into PSUM [1,512];
    sigmoid+bias fused in the ACT eviction; affine+clip epilogue on DVE.
"""

import numpy as np

B = 65536
NCORES = 8
BS = B // NCORES  # 8192 images per core
TN = 512          # images per outer tile
NT = BS // TN     # 16 outer tiles

KDEC = float(np.exp(np.float32(-0.001)))

# set by test.py for profiling; harness leaves these alone
TRACE = False
LAST_EXEC_NS = None
LAST_PROFILE_JSON = None


def _build_weight_mats(W1, b1, W2, b2, Wfc):
    """Host-side restructuring of the tiny conv/fc weights into the
    padded banded matrices the PE matmuls consume."""
    W1 = np.asarray(W1, np.float32).reshape(8, 1, 3, 3)
    W2 = np.asarray(W2, np.float32).reshape(16, 8, 3, 3)
    Wfc = np.asarray(Wfc, np.float32).reshape(1, 784)

    # W1row[(di,j), (co,oj)] over a 3-row x 28-col input window
    W1row = np.zeros((84, 112), np.float32)
    for co in range(8):
        for oj in range(14):
            m = co * 14 + oj
            for di in range(3):
                for dj in range(3):
                    j = 2 * oj - 1 + dj
                    if 0 <= j < 28:
                        W1row[di * 28 + j, m] = W1[co, 0, di, dj]
    # window alignment against 112-pixel (4-row) chunks
    Wa = np.zeros((112, 112), np.float32)
    Wa[0:56] = W1row[28:84]     # window rows 1,2 land at chunk rows 0,1 (oi=2c)
    Wb = np.zeros((112, 112), np.float32)
    Wb[28:112] = W1row[0:84]    # full window at chunk rows 1..3 (oi=2c+1)
    Wc = np.zeros((112, 112), np.float32)
    Wc[84:112] = W1row[0:28]    # window row 0 at chunk row 3 (oi=2c+2)

    # conv2 tap matrices: W2r[di][(ci,j), (co2,oj2)]
    W2r = np.zeros((3, 112, 112), np.float32)
    for di in range(3):
        for co in range(16):
            for oj in range(7):
                m = co * 7 + oj
                for ci in range(8):
                    for dj in range(3):
                        j = 2 * oj - 1 + dj
                        if 0 <= j < 14:
                            W2r[di, ci * 14 + j, m] = W2[co, ci, di, dj]

    # fc columns per z2 row: wfc[(co2,oj2), i2]
    wfc = np.zeros((112, 7), np.float32)
    for co in range(16):
        for i2 in range(7):
            for oj in range(7):
                wfc[co * 7 + oj, i2] = Wfc[0, co * 49 + i2 * 7 + oj]

    b1col = np.repeat(np.asarray(b1, np.float32), 14).reshape(112, 1)
    b2col = np.repeat(np.asarray(b2, np.float32), 7).reshape(112, 1)
    return Wa, Wb, Wc, W2r, wfc, b1col, b2col


def _build_nc(nt_tiles):
    import concourse.bacc as bacc
    import concourse.bass as bass
    import concourse.mybir as mybir
    import concourse.tile as tile
    from concourse.masks import make_identity

    f32 = mybir.dt.float32
    f32r = mybir.dt.float32r
    AF = mybir.ActivationFunctionType
    OP = mybir.AluOpType
    bs = nt_tiles * TN

    nc = bacc.Bacc(None)
    x_d = nc.declare_dram_parameter("x", [bs, 784], f32r, isOutput=False)
    w1a_d = nc.declare_dram_parameter("w1a", [112, 112], f32r, isOutput=False)
    w1b_d = nc.declare_dram_parameter("w1b", [112, 112], f32r, isOutput=False)
    w1c_d = nc.declare_dram_parameter("w1c", [112, 112], f32r, isOutput=False)
    w2r0_d = nc.declare_dram_parameter("w2r0", [112, 112], f32r, isOutput=False)
    w2r1_d = nc.declare_dram_parameter("w2r1", [112, 112], f32r, isOutput=False)
    w2r2_d = nc.declare_dram_parameter("w2r2", [112, 112], f32r, isOutput=False)
    wfc_d = nc.declare_dram_parameter("wfc", [112, 7], f32r, isOutput=False)
    b1_d = nc.declare_dram_parameter("b1col", [112, 1], f32, isOutput=False)
    b2_d = nc.declare_dram_parameter("b2col", [112, 1], f32, isOutput=False)
    bfc_d = nc.declare_dram_parameter("bfc", [1, 1], f32, isOutput=False)
    y_d = nc.declare_dram_parameter("y", [bs], f32, isOutput=True)

    with tile.TileContext(nc) as tc:
        with (
            tc.tile_pool(name="const", bufs=1) as const,
            tc.tile_pool(name="xs_pool", bufs=2) as xs_pool,
            tc.tile_pool(name="xt_pool", bufs=5) as xt_pool,
            tc.tile_pool(name="z1_pool", bufs=8) as z1_pool,
            tc.tile_pool(name="z2_pool", bufs=5) as z2_pool,
            tc.tile_pool(name="y_pool", bufs=1) as y_pool,
            tc.tile_pool(name="tp_psum", bufs=2, space="PSUM") as tp_pool,
            tc.tile_pool(name="c1_psum", bufs=3, space="PSUM") as c1_pool,
            tc.tile_pool(name="c2_psum", bufs=2, space="PSUM") as c2_pool,
            tc.tile_pool(name="fc_psum", bufs=1, space="PSUM") as fc_pool,
        ):
            ident_f32 = const.tile([128, 128], f32, name="ident_f32")
            make_identity(nc, ident_f32)
            ident = const.tile([128, 128], f32r, name="ident")
            nc.vector.tensor_copy(ident[:], ident_f32[:])
            w1a = const.tile([112, 112], f32r, tag="w1a")
            w1b = const.tile([112, 112], f32r, tag="w1b")
            w1c = const.tile([112, 112], f32r, tag="w1c")
            w2r0 = const.tile([112, 112], f32r, tag="w2r0")
            w2r1 = const.tile([112, 112], f32r, tag="w2r1")
            w2r2 = const.tile([112, 112], f32r, tag="w2r2")
            wfc = const.tile([112, 7], f32r, tag="wfc")
            b1 = const.tile([112, 1], f32, tag="b1")
            b2 = const.tile([112, 1], f32, tag="b2")
            bfc = const.tile([1, 1], f32, tag="bfc")
            for sb, dr in [(w1a, w1a_d), (w1b, w1b_d), (w1c, w1c_d),
                           (w2r0, w2r0_d), (w2r1, w2r1_d), (w2r2, w2r2_d),
                           (wfc, wfc_d), (b1, b1_d), (b2, b2_d), (bfc, bfc_d)]:
                nc.sync.dma_start(out=sb[:], in_=dr[:])

            # single-partition staging laid out in DRAM byte order
            y_sb = y_pool.tile([1, nt_tiles * TN], f32)

            # round-robin the PSUM->SBUF evictions across ACT and DVE
            evict_i = [0]

            def evict_copy(dst, src):
                evict_i[0] += 1
                if evict_i[0] % 2:
                    nc.vector.tensor_copy(dst, src)
                else:
                    nc.scalar.copy(dst, src)

            def evict_relu(dst, src, bias):
                evict_i[0] += 1
                if evict_i[0] % 2:
                    nc.vector.tensor_scalar(dst, src, bias[:, 0:1], 0.0,
                                            OP.add, OP.max)
                else:
                    nc.scalar.activation(dst, src, AF.Relu, bias=bias[:, 0:1])

            for t in range(nt_tiles):
                xs = xs_pool.tile([128, 3136], f32r, tag="xs", name="xs")
                nc.sync.dma_start(
                    out=xs[:],
                    in_=x_d[bass.ds(t * TN, TN), :].rearrange(
                        "(p s) f -> p (s f)", s=4),
                )
                p1 = {}
                p2 = {}
                z1 = {}
                z2 = {}
                fcp = None
                for c in range(7):
                    tp = tp_pool.tile([112, TN], f32r, tag="tp", name="tp")
                    for s in range(4):
                        nc.tensor.transpose(
                            tp[:, bass.ts(s, 128)],
                            xs[:, s * 784 + c * 112: s * 784 + (c + 1) * 112],
                            ident,
                        )
                    xt = xt_pool.tile([112, TN], f32r, tag="xt", name="xt")
                    evict_copy(xt[:], tp[:])

                    # ---- conv1 rows fed by chunk c ----
                    oi = 2 * c
                    if c == 0:
                        p1[0] = c1_pool.tile([112, TN], f32, tag="p1", name="p1")
                        nc.tensor.matmul(p1[0][:], w1a[:], xt[:],
                                         start=True, stop=True)
                    else:
                        nc.tensor.matmul(p1[oi][:], w1a[:], xt[:],
                                         start=False, stop=True)
                    z1[oi] = z1_pool.tile([112, TN], f32r, tag="z1", name="z1")
                    evict_relu(z1[oi][:], p1[oi][:], b1)

                    oi = 2 * c + 1
                    p1[oi] = c1_pool.tile([112, TN], f32, tag="p1", name="p1")
                    nc.tensor.matmul(p1[oi][:], w1b[:], xt[:],
                                     start=True, stop=True)
                    z1[oi] = z1_pool.tile([112, TN], f32r, tag="z1", name="z1")
                    evict_relu(z1[oi][:], p1[oi][:], b1)

                    if c < 6:
                        p1[2 * c + 2] = c1_pool.tile([112, TN], f32, tag="p1", name="p1")
                        nc.tensor.matmul(p1[2 * c + 2][:], w1c[:], xt[:],
                                         start=True, stop=False)

                    # ---- conv2 rows 2c, 2c+1 of z1 ----
                    for r in (2 * c, 2 * c + 1):
                        if r % 2 == 0:
                            oi2 = r // 2
                            if oi2 == 0:
                                p2[0] = c2_pool.tile([112, TN], f32, tag="p2", name="p2")
                                nc.tensor.matmul(p2[0][:], w2r1[:], z1[0][:],
                                                 start=True, stop=False)
                            else:
                                nc.tensor.matmul(p2[oi2][:], w2r1[:], z1[r][:],
                                                 start=False, stop=False)
                        else:
                            lo = (r - 1) // 2
                            nc.tensor.matmul(p2[lo][:], w2r2[:], z1[r][:],
                                             start=False, stop=True)
                            hi = (r + 1) // 2
                            if hi <= 6:
                                p2[hi] = c2_pool.tile([112, TN], f32, tag="p2", name="p2")
                                nc.tensor.matmul(p2[hi][:], w2r0[:], z1[r][:],
                                                 start=True, stop=False)
                            # p2[lo] complete -> relu -> fc
                            z2[lo] = z2_pool.tile([112, TN], f32r, tag="z2", name="z2")
                            evict_relu(z2[lo][:], p2[lo][:], b2)
                            if lo == 0:
                                fcp = fc_pool.tile([1, TN], f32, tag="fc", name="fc")
                            nc.tensor.matmul(fcp[:], wfc[:, lo:lo + 1],
                                             z2[lo][:],
                                             start=(lo == 0), stop=(lo == 6))

                # scatter into DRAM order: column c=(s*128+p) -> offset 4p+s
                y_slice = y_sb[0:1, bass.ds(t * TN, TN)].rearrange(
                    "q (p s) -> q s p", p=128, s=4)
                nc.scalar.activation(y_slice, fcp[:], AF.Sigmoid,
                                     bias=bfc[:, 0:1])
                # p = (1-k) + k*sigmoid(t); clip is a mathematical no-op
                nc.vector.tensor_scalar(y_sb[0:1, bass.ds(t * TN, TN)],
                                        y_sb[0:1, bass.ds(t * TN, TN)],
                                        KDEC, 1.0 - KDEC, OP.mult, OP.add)
                nc.vector.tensor_scalar(y_sb[0:1, bass.ds(t * TN, TN)],
                                        y_sb[0:1, bass.ds(t * TN, TN)],
                                        1e-6, 1.0 - 1e-6, OP.max, OP.min)

            nc.sync.dma_start(out=y_d[:], in_=y_sb[0:1, :])

    nc.finalize()
    return nc


_NC_CACHE = {}


def _get_nc(nt_tiles):
    if nt_tiles not in _NC_CACHE:
        _NC_CACHE[nt_tiles] = _build_nc(nt_tiles)
    return _NC_CACHE[nt_tiles]


def _install_trace_hook():
    """Register the axon NTFF profiling hook (test-time only)."""
    import contextlib
    import ctypes
    import sys
    import types

    if "antenv.axon_hooks" in sys.modules:
        return
    try:
        lib = ctypes.CDLL("/opt/axon/libaxon_pjrt.so")
        if not hasattr(lib, "axon_start_nrt_profile"):
            return
        lib.axon_start_nrt_profile.argtypes = [
            ctypes.POINTER(ctypes.c_int64), ctypes.c_size_t]
        lib.axon_start_nrt_profile.restype = ctypes.c_int64
        lib.axon_stop_nrt_profile.argtypes = [ctypes.c_char_p]
        lib.axon_stop_nrt_profile.restype = ctypes.c_int64

        @contextlib.contextmanager
        def _hook(output_dir, device_ids):
            import jax
            jax.devices()
            if device_ids:
                ids = (ctypes.c_int64 * len(device_ids))(*device_ids)
                rc = lib.axon_start_nrt_profile(ids, len(device_ids))
            else:
                rc = lib.axon_start_nrt_profile(None, 0)
            if rc != 0:
                raise RuntimeError(f"axon_start_nrt_profile rc={rc}")
            try:
                yield
            finally:
                rc = lib.axon_stop_nrt_profile(output_dir.encode())
                if rc not in (0, 3):
                    raise RuntimeError(f"axon_stop_nrt_profile rc={rc}")

        mod = types.ModuleType("antenv.axon_hooks")
        mod.get_axon_ntff_profile_hook = lambda: _hook
        mod.set_axon_ntff_profile_hook = lambda h: None
        sys.modules["antenv.axon_hooks"] = mod
        import concourse.bass_utils as bu
        bu.upload_artifacts = lambda tmpdir: tmpdir
    except Exception:
        pass


def kernel(x, W1, b1, W2, b2, Wfc, bfc):
    global LAST_EXEC_NS, LAST_PROFILE_JSON
    from concourse.bass_utils import run_bass_kernel_spmd

    x = np.ascontiguousarray(np.asarray(x, np.float32).reshape(B, 784))
    Wa, Wb, Wc, W2r, wfc, b1col, b2col = _build_weight_mats(W1, b1, W2, b2, Wfc)
    bfc_a = np.asarray(bfc, np.float32).reshape(1, 1)

    nc = _get_nc(NT)
    shared = {
        "w1a": Wa, "w1b": Wb, "w1c": Wc,
        "w2r0": np.ascontiguousarray(W2r[0]),
        "w2r1": np.ascontiguousarray(W2r[1]),
        "w2r2": np.ascontiguousarray(W2r[2]),
        "wfc": wfc, "b1col": b1col, "b2col": b2col, "bfc": bfc_a,
    }
    in_maps = [
        {"x": x[i * BS:(i + 1) * BS], **shared} for i in range(NCORES)
    ]
    core_ids = list(range(NCORES))
    res = run_bass_kernel_spmd(nc, in_maps, core_ids)
    y = np.concatenate([res.results[i]["y"] for i in range(NCORES)])

    if TRACE:
        _install_trace_hook()
        try:
            tres = run_bass_kernel_spmd(nc, in_maps, core_ids, trace=True)
            LAST_EXEC_NS = tres.exec_time_ns
            LAST_PROFILE_JSON = tres.profile_json
        except Exception as e:  # profiling must never break the result path
            print("trace failed:", e)

    return y.astype(np.float32)


# revision 9
# speedup vs baseline: 1.0733x; 1.0733x over previous
"""Trainium2 Bass kernel for the DiffsolClassifier model.

Network (per image, NCHW fp32):
    z1 = relu(conv2d(x, W1, b1, k=3, s=2, p=1))   # [8,14,14]
    z2 = relu(conv2d(z1, W2, b2, k=3, s=2, p=1))  # [16,7,7]
    t  = flatten(z2) @ Wfc.T + bfc                # [1]
    p  = clip(1 - exp(-(softplus(t) + 1e-3)), 1e-6, 1-1e-6)
       = (1-k) + k*sigmoid(t),  k = exp(-1e-3)    (clip is a no-op)

Sharding: pure data parallel, batch 65536 split 8192/core across 8 cores.

Per-core mapping (16 outer tiles x 512 images):
  - DMA x tile [512, 784] -> SBUF [128, 3136] (partition p holds images 4p+s).
  - PE-transpose 112-pixel chunks to pixel-major [112, 512] (7 chunks).
  - conv1: each output row oi contracts an 84-pixel window; windows are
    expressed against the aligned 112-pixel chunks with zero-padded
    [112,112] weight matrices (Wa/Wb/Wc), N=512 fp32r matmuls accumulating
    in PSUM. Bias+relu fused into the PSUM->SBUF eviction (ACT/DVE).
  - conv2: z1 row-chunks [112=(ch,col), 512] contract with banded
    [112,112] tap matrices (3 taps -> 3 accumulated matmuls per out row).
  - FC: per out-row [112,1] matmul accumulated into PSUM [1,512];
    sigmoid+bias fused in the ACT eviction; affine+clip epilogue on DVE.
"""

import numpy as np

B = 65536
NCORES = 8
BS = B // NCORES  # 8192 images per core
TN = 512          # images per outer tile
NT = BS // TN     # 16 outer tiles

KDEC = float(np.exp(np.float32(-0.001)))

# set by test.py for profiling; harness leaves these alone
TRACE = False
LAST_EXEC_NS = None
LAST_PROFILE_JSON = None


def _build_weight_mats(W1, b1, W2, b2, Wfc):
    """Host-side restructuring of the tiny conv/fc weights into the
    padded banded matrices the PE matmuls consume."""
    W1 = np.asarray(W1, np.float32).reshape(8, 1, 3, 3)
    W2 = np.asarray(W2, np.float32).reshape(16, 8, 3, 3)
    Wfc = np.asarray(Wfc, np.float32).reshape(1, 784)

    # W1row[(di,j), (co,oj)] over a 3-row x 28-col input window
    W1row = np.zeros((84, 112), np.float32)
    for co in range(8):
        for oj in range(14):
            m = co * 14 + oj
            for di in range(3):
                for dj in range(3):
                    j = 2 * oj - 1 + dj
                    if 0 <= j < 28:
                        W1row[di * 28 + j, m] = W1[co, 0, di, dj]
    # window alignment against 112-pixel (4-row) chunks
    Wa = np.zeros((112, 112), np.float32)
    Wa[0:56] = W1row[28:84]     # window rows 1,2 land at chunk rows 0,1 (oi=2c)
    Wb = np.zeros((112, 112), np.float32)
    Wb[28:112] = W1row[0:84]    # full window at chunk rows 1..3 (oi=2c+1)
    Wc = np.zeros((112, 112), np.float32)
    Wc[84:112] = W1row[0:28]    # window row 0 at chunk row 3 (oi=2c+2)

    # conv2 tap matrices: W2r[di][(ci,j), (co2,oj2)]
    W2r = np.zeros((3, 112, 112), np.float32)
    for di in range(3):
        for co in range(16):
            for oj in range(7):
                m = co * 7 + oj
                for ci in range(8):
                    for dj in range(3):
                        j = 2 * oj - 1 + dj
                        if 0 <= j < 14:
                            W2r[di, ci * 14 + j, m] = W2[co, ci, di, dj]

    # fc columns per z2 row: wfc[(co2,oj2), i2]
    wfc = np.zeros((112, 7), np.float32)
    for co in range(16):
        for i2 in range(7):
            for oj in range(7):
                wfc[co * 7 + oj, i2] = Wfc[0, co * 49 + i2 * 7 + oj]

    b1col = np.repeat(np.asarray(b1, np.float32), 14).reshape(112, 1)
    b2col = np.repeat(np.asarray(b2, np.float32), 7).reshape(112, 1)
    return Wa, Wb, Wc, W2r, wfc, b1col, b2col


def _build_nc(nt_tiles):
    import concourse.bacc as bacc
    import concourse.bass as bass
    import concourse.mybir as mybir
    import concourse.tile as tile
    from concourse.masks import make_identity

    f32 = mybir.dt.float32
    f32r = mybir.dt.float32r
    f16 = mybir.dt.float16
    AF = mybir.ActivationFunctionType
    OP = mybir.AluOpType
    bs = nt_tiles * TN

    nc = bacc.Bacc(None)
    x_d = nc.declare_dram_parameter("x", [bs, 784], f32, isOutput=False)
    w1a_d = nc.declare_dram_parameter("w1a", [112, 112], f16, isOutput=False)
    w1b_d = nc.declare_dram_parameter("w1b", [112, 112], f16, isOutput=False)
    w1c_d = nc.declare_dram_parameter("w1c", [112, 112], f16, isOutput=False)
    w2r0_d = nc.declare_dram_parameter("w2r0", [112, 112], f16, isOutput=False)
    w2r1_d = nc.declare_dram_parameter("w2r1", [112, 112], f16, isOutput=False)
    w2r2_d = nc.declare_dram_parameter("w2r2", [112, 112], f16, isOutput=False)
    wfc_d = nc.declare_dram_parameter("wfc", [112, 7], f16, isOutput=False)
    b1_d = nc.declare_dram_parameter("b1col", [112, 1], f32, isOutput=False)
    b2_d = nc.declare_dram_parameter("b2col", [112, 1], f32, isOutput=False)
    bfc_d = nc.declare_dram_parameter("bfc", [1, 1], f32, isOutput=False)
    y_d = nc.declare_dram_parameter("y", [bs], f32, isOutput=True)

    with tile.TileContext(nc) as tc:
        with (
            tc.tile_pool(name="const", bufs=1) as const,
            tc.tile_pool(name="xs_pool", bufs=2) as xs_pool,
            tc.tile_pool(name="xt_pool", bufs=5) as xt_pool,
            tc.tile_pool(name="z1_pool", bufs=8) as z1_pool,
            tc.tile_pool(name="z2_pool", bufs=5) as z2_pool,
            tc.tile_pool(name="y_pool", bufs=1) as y_pool,
            tc.tile_pool(name="tp_psum", bufs=2, space="PSUM") as tp_pool,
            tc.tile_pool(name="c1_psum", bufs=3, space="PSUM") as c1_pool,
            tc.tile_pool(name="c2_psum", bufs=2, space="PSUM") as c2_pool,
            tc.tile_pool(name="fc_psum", bufs=1, space="PSUM") as fc_pool,
        ):
            ident_f32 = const.tile([128, 128], f32, name="ident_f32")
            make_identity(nc, ident_f32)
            ident = const.tile([128, 128], f16, name="ident")
            nc.vector.tensor_copy(ident[:], ident_f32[:])
            w1a = const.tile([112, 112], f16, tag="w1a")
            w1b = const.tile([112, 112], f16, tag="w1b")
            w1c = const.tile([112, 112], f16, tag="w1c")
            w2r0 = const.tile([112, 112], f16, tag="w2r0")
            w2r1 = const.tile([112, 112], f16, tag="w2r1")
            w2r2 = const.tile([112, 112], f16, tag="w2r2")
            wfc = const.tile([112, 7], f16, tag="wfc")
            b1 = const.tile([112, 1], f32, tag="b1")
            b2 = const.tile([112, 1], f32, tag="b2")
            bfc = const.tile([1, 1], f32, tag="bfc")
            for sb, dr in [(w1a, w1a_d), (w1b, w1b_d), (w1c, w1c_d),
                           (w2r0, w2r0_d), (w2r1, w2r1_d), (w2r2, w2r2_d),
                           (wfc, wfc_d), (b1, b1_d), (b2, b2_d), (bfc, bfc_d)]:
                nc.sync.dma_start(out=sb[:], in_=dr[:])

            # single-partition staging laid out in DRAM byte order
            y_sb = y_pool.tile([1, nt_tiles * TN], f32)

            # round-robin the PSUM->SBUF evictions across ACT and DVE
            evict_i = [0]

            def evict_copy(dst, src):
                evict_i[0] += 1
                if evict_i[0] % 2:
                    nc.vector.tensor_copy(dst, src)
                else:
                    nc.scalar.copy(dst, src)

            def evict_relu(dst, src, bias):
                evict_i[0] += 1
                if evict_i[0] % 2:
                    nc.vector.tensor_scalar(dst, src, bias[:, 0:1], 0.0,
                                            OP.add, OP.max)
                else:
                    nc.scalar.activation(dst, src, AF.Relu, bias=bias[:, 0:1])

            for t in range(nt_tiles):
                xs = xs_pool.tile([128, 3136], f16, tag="xs", name="xs")
                nc.gpsimd.dma_start(
                    out=xs[:],
                    in_=x_d[bass.ds(t * TN, TN), :].rearrange(
                        "(p s) f -> p (s f)", s=4),
                )
                p1 = {}
                p2 = {}
                z1 = {}
                z2 = {}
                fcp = None
                for c in range(7):
                    tp = tp_pool.tile([112, TN], f16, tag="tp", name="tp")
                    for s in range(4):
                        nc.tensor.transpose(
                            tp[:, bass.ts(s, 128)],
                            xs[:, s * 784 + c * 112: s * 784 + (c + 1) * 112],
                            ident,
                        )
                    xt = xt_pool.tile([112, TN], f16, tag="xt", name="xt")
                    evict_copy(xt[:], tp[:])

                    # ---- conv1 rows fed by chunk c ----
                    oi = 2 * c
                    if c == 0:
                        p1[0] = c1_pool.tile([112, TN], f32, tag="p1", name="p1")
                        nc.tensor.matmul(p1[0][:], w1a[:], xt[:],
                                         start=True, stop=True)
                    else:
                        nc.tensor.matmul(p1[oi][:], w1a[:], xt[:],
                                         start=False, stop=True)
                    z1[oi] = z1_pool.tile([112, TN], f16, tag="z1", name="z1")
                    evict_relu(z1[oi][:], p1[oi][:], b1)

                    oi = 2 * c + 1
                    p1[oi] = c1_pool.tile([112, TN], f32, tag="p1", name="p1")
                    nc.tensor.matmul(p1[oi][:], w1b[:], xt[:],
                                     start=True, stop=True)
                    z1[oi] = z1_pool.tile([112, TN], f16, tag="z1", name="z1")
                    evict_relu(z1[oi][:], p1[oi][:], b1)

                    if c < 6:
                        p1[2 * c + 2] = c1_pool.tile([112, TN], f32, tag="p1", name="p1")
                        nc.tensor.matmul(p1[2 * c + 2][:], w1c[:], xt[:],
                                         start=True, stop=False)

                    # ---- conv2 rows 2c, 2c+1 of z1 ----
                    for r in (2 * c, 2 * c + 1):
                        if r % 2 == 0:
                            oi2 = r // 2
                            if oi2 == 0:
                                p2[0] = c2_pool.tile([112, TN], f32, tag="p2", name="p2")
                                nc.tensor.matmul(p2[0][:], w2r1[:], z1[0][:],
                                                 start=True, stop=False)
                            else:
                                nc.tensor.matmul(p2[oi2][:], w2r1[:], z1[r][:],
                                                 start=False, stop=False)
                        else:
                            lo = (r - 1) // 2
                            nc.tensor.matmul(p2[lo][:], w2r2[:], z1[r][:],
                                             start=False, stop=True)
                            hi = (r + 1) // 2
                            if hi <= 6:
                                p2[hi] = c2_pool.tile([112, TN], f32, tag="p2", name="p2")
                                nc.tensor.matmul(p2[hi][:], w2r0[:], z1[r][:],
                                                 start=True, stop=False)
                            # p2[lo] complete -> relu -> fc
                            z2[lo] = z2_pool.tile([112, TN], f16, tag="z2", name="z2")
                            evict_relu(z2[lo][:], p2[lo][:], b2)
                            if lo == 0:
                                fcp = fc_pool.tile([1, TN], f32, tag="fc", name="fc")
                            nc.tensor.matmul(fcp[:], wfc[:, lo:lo + 1],
                                             z2[lo][:],
                                             start=(lo == 0), stop=(lo == 6))

                # scatter into DRAM order: column c=(s*128+p) -> offset 4p+s
                y_slice = y_sb[0:1, bass.ds(t * TN, TN)].rearrange(
                    "q (p s) -> q s p", p=128, s=4)
                nc.scalar.activation(y_slice, fcp[:], AF.Sigmoid,
                                     bias=bfc[:, 0:1])
                # p = (1-k) + k*sigmoid(t); clip is a mathematical no-op
                nc.vector.tensor_scalar(y_sb[0:1, bass.ds(t * TN, TN)],
                                        y_sb[0:1, bass.ds(t * TN, TN)],
                                        KDEC, 1.0 - KDEC, OP.mult, OP.add)
                nc.vector.tensor_scalar(y_sb[0:1, bass.ds(t * TN, TN)],
                                        y_sb[0:1, bass.ds(t * TN, TN)],
                                        1e-6, 1.0 - 1e-6, OP.max, OP.min)

            nc.sync.dma_start(out=y_d[:], in_=y_sb[0:1, :])

    nc.finalize()
    return nc


_NC_CACHE = {}


def _get_nc(nt_tiles):
    if nt_tiles not in _NC_CACHE:
        _NC_CACHE[nt_tiles] = _build_nc(nt_tiles)
    return _NC_CACHE[nt_tiles]


def _install_trace_hook():
    """Register the axon NTFF profiling hook (test-time only)."""
    import contextlib
    import ctypes
    import sys
    import types

    if "antenv.axon_hooks" in sys.modules:
        return
    try:
        lib = ctypes.CDLL("/opt/axon/libaxon_pjrt.so")
        if not hasattr(lib, "axon_start_nrt_profile"):
            return
        lib.axon_start_nrt_profile.argtypes = [
            ctypes.POINTER(ctypes.c_int64), ctypes.c_size_t]
        lib.axon_start_nrt_profile.restype = ctypes.c_int64
        lib.axon_stop_nrt_profile.argtypes = [ctypes.c_char_p]
        lib.axon_stop_nrt_profile.restype = ctypes.c_int64

        @contextlib.contextmanager
        def _hook(output_dir, device_ids):
            import jax
            jax.devices()
            if device_ids:
                ids = (ctypes.c_int64 * len(device_ids))(*device_ids)
                rc = lib.axon_start_nrt_profile(ids, len(device_ids))
            else:
                rc = lib.axon_start_nrt_profile(None, 0)
            if rc != 0:
                raise RuntimeError(f"axon_start_nrt_profile rc={rc}")
            try:
                yield
            finally:
                rc = lib.axon_stop_nrt_profile(output_dir.encode())
                if rc not in (0, 3):
                    raise RuntimeError(f"axon_stop_nrt_profile rc={rc}")

        mod = types.ModuleType("antenv.axon_hooks")
        mod.get_axon_ntff_profile_hook = lambda: _hook
        mod.set_axon_ntff_profile_hook = lambda h: None
        sys.modules["antenv.axon_hooks"] = mod
        import concourse.bass_utils as bu
        bu.upload_artifacts = lambda tmpdir: tmpdir
    except Exception:
        pass


def kernel(x, W1, b1, W2, b2, Wfc, bfc):
    global LAST_EXEC_NS, LAST_PROFILE_JSON
    from concourse.bass_utils import run_bass_kernel_spmd

    x = np.ascontiguousarray(np.asarray(x, np.float32).reshape(B, 784))
    Wa, Wb, Wc, W2r, wfc, b1col, b2col = _build_weight_mats(W1, b1, W2, b2, Wfc)
    bfc_a = np.asarray(bfc, np.float32).reshape(1, 1)

    nc = _get_nc(NT)
    shared = {
        "w1a": Wa.astype(np.float16), "w1b": Wb.astype(np.float16),
        "w1c": Wc.astype(np.float16),
        "w2r0": np.ascontiguousarray(W2r[0]).astype(np.float16),
        "w2r1": np.ascontiguousarray(W2r[1]).astype(np.float16),
        "w2r2": np.ascontiguousarray(W2r[2]).astype(np.float16),
        "wfc": wfc.astype(np.float16),
        "b1col": b1col, "b2col": b2col, "bfc": bfc_a,
    }
    in_maps = [
        {"x": x[i * BS:(i + 1) * BS], **shared} for i in range(NCORES)
    ]
    core_ids = list(range(NCORES))
    res = run_bass_kernel_spmd(nc, in_maps, core_ids)
    y = np.concatenate([res.results[i]["y"] for i in range(NCORES)])

    if TRACE:
        _install_trace_hook()
        try:
            tres = run_bass_kernel_spmd(nc, in_maps, core_ids, trace=True)
            LAST_EXEC_NS = tres.exec_time_ns
            LAST_PROFILE_JSON = tres.profile_json
        except Exception as e:  # profiling must never break the result path
            print("trace failed:", e)

    return y.astype(np.float32)
